# revision 21
# baseline (speedup 1.0000x reference)
"""Trainium2 kernel for nn_MemoryAttentionLayer (retrieval_knn) — v2.

Strategy (shard memory rows across 8 cores, replicate queries):

Device (8 cores, SPMD — the full 34.4 GFLOP score scan):
  - each core holds a row-shard of memory_keys as keysT [kd=128, slots]
    in FP8 e4m3 (scaled x8) — halves HBM traffic vs bf16 (48us vs 96us
    at 358 GB/s/core); fp8 matmuls stream at the same 1 col/cycle.
  - PE: stationary qT [kd, 128 queries] fp8 (+~4us HAM warmup burst),
    256 matmuls N=512 -> scores (x8 scale) in PSUM fp32, grouped into
    128 units of 1024 slots = 2 PSUM banks. Four units in flight with
    explicit round-robin slot tags (the pool's default MRU slot reuse
    would serialize each engine into consume->refill->consume).
    Redundant per-matmul LDWEIGHTS are deduped post-build.
  - drain is split between the only two engines that can read PSUM
    (GPSIMD/DMA physically cannot; TT with 2 PSUM operands is illegal):
      * DVE units (17/32): reduce_max over v=64 -> rowmax[q, row]
        (row-granular flags), (1024+140)/0.96 ~ 1.21us per unit.
      * ACT units (15/32): activation(Relu, bias=-(8*t_f - margin),
        accum_out) + accumulator readout -> hinge[q, unit] > 0 iff the
        unit has a slot above threshold (16-row flags), ~1.41us per unit.
    ACT output goes to an SBUF scratch tile: pointing it at PSUM slows
    concurrent matmuls ~300x (measured).
Host (tail, exact):
  - query projection, per-query sigma -> flag threshold t_f = Z*sigma,
  - union of flagged rows -> ONE sgemm exact fp32 rescore for all
    queries x union-rows, per-query top-32 with count-check fallback
    (full rescore) so top-k is exact regardless of fp8/threshold error,
  - attention softmax over 32, update matmul, scatter-add, layer norm.
"""

import os
import numpy as np
import ml_dtypes

bf16 = ml_dtypes.bfloat16
f8e4 = ml_dtypes.float8_e4m3

# ---- problem constants (hardcoded per spec) ----
N_CORES = 8
B, T, H = 4, 512, 768
NM = 128                      # n_mentions / queries
ROWS, VPR, KD = 16384, 64, 128
K_TOP = 32
LN_EPS = 1e-12

NSLOTS = ROWS * VPR           # 1048576
SPC = NSLOTS // N_CORES       # 131072 slots per core
RPC = ROWS // N_CORES         # 2048 rows per core
MMN = 512                     # moving cols per matmul (1 fp32 PSUM bank)
UNIT = int(os.environ.get("MK_UNIT", 1024))  # slots per drain unit
NUNITS = SPC // UNIT          # units per core
ROWS_PER_UNIT = UNIT // VPR
KSCALE = 8.0                  # fp8 key scale; PSUM scores are 8*s

# ---- tunables ----
CHUNK = int(os.environ.get("MK_CHUNK", 16384))      # slots per DMA chunk
N_DVE_OF_32 = int(os.environ.get("MK_NDVE", 17))    # DVE units per 32
Z_FLAG = float(os.environ.get("MK_Z", 3.8))         # flag threshold z
M_Z = float(os.environ.get("MK_MZ", 0.30))          # fp8 margin, in sigmas
HINGE_INPLACE = os.environ.get("MK_HIP", "0") == "1"
# key path: f8cast = fp8 HBM + DMA-cast to bf16 SBUF; bf16 = bf16 HBM;
# f8mix = bf16 queries x fp8 keys matmul; f8mm = fp8 x fp8 matmul
KEYS_MODE = os.environ.get("MK_KEYS", "f8mm")
LDW_OPT = os.environ.get("MK_LDWOPT", "0") == "1"


def _maybe_patch_ldwopt():
    """Flip walrus --enable-ldw-opt when MK_LDWOPT=1 (A/B experiment)."""
    if not LDW_OPT:
        return
    import concourse.bass_utils as _bu
    if getattr(_bu, "_mk_ldwopt_patch", False):
        return
    _orig_rc = _bu.run_command

    def _rc(cmd, **kw):
        cmd = [("--enable-ldw-opt=true" if c == "--enable-ldw-opt=false" else c)
               for c in cmd]
        return _orig_rc(cmd, **kw)

    _bu.run_command = _rc
    _bu._mk_ldwopt_patch = True

_NC_CACHE: dict = {}


def _dve_units(nunits, n_dve_of_32):
    """Which unit indices drain via DVE reduce_max (rest via ACT hinge)."""
    return {u for u in range(nunits)
            if (u * n_dve_of_32) % 32 < n_dve_of_32}


def _build_nc(spc=SPC, chunk=CHUNK, n_dve_of_32=None, hinge_inplace=None):
    import concourse.bacc as bacc
    import concourse.mybir as mybir
    from concourse import tile

    if n_dve_of_32 is None:
        n_dve_of_32 = N_DVE_OF_32
    if hinge_inplace is None:
        hinge_inplace = HINGE_INPLACE

    nunits = spc // UNIT
    mm_per_unit = UNIT // MMN                      # 4
    # ramped chunk plan: tiny first chunk so the PE starts ~5us earlier
    # (a full 16K-slot fp8 chunk is ~6us of DMA before the first matmul).
    chunks = []
    pos = 0
    for sz in (2048, 6144, 8192):
        if pos + sz <= spc and sz <= chunk:
            chunks.append((pos, sz))
            pos += sz
    while pos < spc:
        sz = min(chunk, spc - pos)
        chunks.append((pos, sz))
        pos += sz
    dve_set = _dve_units(nunits, n_dve_of_32)
    n_dve = len(dve_set)
    n_act = nunits - n_dve

    mode = KEYS_MODE
    kdt_dram = (mybir.dt.bfloat16 if mode == "bf16" else mybir.dt.float8e4)
    kdt_sbuf = (mybir.dt.float8e4 if mode in ("f8mm", "f8mix")
                else mybir.dt.bfloat16)
    qdt = mybir.dt.float8e4 if mode == "f8mm" else mybir.dt.bfloat16

    nc = bacc.Bacc()
    keysT_d = nc.dram_tensor("keysT", [KD, spc], kdt_dram,
                             kind="ExternalInput")
    qT_d = nc.dram_tensor("qT", [KD, NM], qdt,
                          kind="ExternalInput")
    tqneg_d = nc.dram_tensor("tqneg", [NM, 1], mybir.dt.float32,
                             kind="ExternalInput")
    rowmax_d = nc.dram_tensor("rowmax", [NM, max(n_dve, 1) * ROWS_PER_UNIT],
                              mybir.dt.float32, kind="ExternalOutput")
    hinge_d = nc.dram_tensor("hinge", [NM, max(n_act, 1)],
                             mybir.dt.float32, kind="ExternalOutput")

    # packed output columns per unit
    rm_col = {}
    hg_col = {}
    for u in range(nunits):
        if u in dve_set:
            rm_col[u] = len(rm_col)
        else:
            hg_col[u] = len(hg_col)

    with tile.TileContext(nc) as tc:
        with (
            tc.tile_pool(name="kpool", bufs=3) as kpool,
            tc.tile_pool(name="const", bufs=1) as const_pool,
            tc.tile_pool(name="outs", bufs=1) as out_pool,
            tc.tile_pool(name="scr", bufs=2) as scr_pool,
            tc.tile_pool(name="ps", bufs=1, space="PSUM") as ps_pool,
        ):
            q_t = const_pool.tile([KD, NM], qdt)
            nc.sync.dma_start(q_t[:], qT_d[:])
            tq_t = const_pool.tile([NM, 1], mybir.dt.float32)
            nc.sync.dma_start(tq_t[:], tqneg_d[:])

            rm_t = out_pool.tile([NM, max(n_dve, 1) * ROWS_PER_UNIT],
                                 mybir.dt.float32)
            hg_t = out_pool.tile([NM, max(n_act, 1)], mybir.dt.float32)

            # consume the tq DMA dep up front (keeps scheduler simple)
            tq_warm = const_pool.tile([NM, 1], mybir.dt.float32)
            nc.scalar.copy(tq_warm[:], tq_t[:])

            # HAM warmup: ~4us of back-to-back matmuls against the query
            # tile while the first keys chunk DMA is in flight, so the PE
            # clock is at 8/8 when the real scan starts (cold MMs run at
            # 1.2GHz, ~375ns vs 216ns warm for N=512). Parked on slot ps3
            # (first used by ACT's second unit, which has slack) so it does
            # not delay DVE's first unit on ps0.
            warm_ps = ps_pool.tile([NM, MMN], mybir.dt.float32, tag="ps3")
            for _ in range(24):
                nc.tensor.matmul(warm_ps[:, 0:NM], q_t[:], q_t[:],
                                 start=True, stop=True)
            if n_act == 0:
                nc.scalar.memzero(hg_t[:])
            if n_dve == 0:
                nc.scalar.memzero(rm_t[:])

            nslots = 8 * MMN // UNIT
            for cstart, csize in chunks:
                k_t = kpool.tile([KD, csize], kdt_sbuf, tag="k_t")
                if mode == "f8cast":
                    nc.gpsimd.dma_start(
                        k_t[:], keysT_d[:, cstart:cstart + csize])
                else:
                    nc.sync.dma_start(
                        k_t[:], keysT_d[:, cstart:cstart + csize])
                for ui in range(csize // UNIT):
                    u = (cstart + ui * UNIT) // UNIT
                    # explicit slot tag: strict round-robin over the PSUM
                    # banks so each engine's next unit is pre-filled while
                    # it processes the current one (the pool's default slot
                    # reuse is most-recently-freed, which serializes
                    # consume -> refill -> consume per engine).
                    ps = ps_pool.tile([NM, UNIT], mybir.dt.float32,
                                      tag=f"ps{u % nslots}")
                    for mi in range(mm_per_unit):
                        off = ui * UNIT + mi * MMN
                        nc.tensor.matmul(ps[:, mi * MMN:(mi + 1) * MMN],
                                         q_t[:],
                                         k_t[:, off:off + MMN],
                                         start=True, stop=True)
                    if u in dve_set:
                        o0 = rm_col[u] * ROWS_PER_UNIT
                        nc.vector.reduce_max(
                            rm_t[:, o0:o0 + ROWS_PER_UNIT],
                            ps[:].rearrange("p (r v) -> p r v", v=VPR),
                            axis=mybir.AxisListType.X)
                    else:
                        if hinge_inplace:
                            out_ap = ps[:]
                        else:
                            scr = scr_pool.tile([NM, UNIT], mybir.dt.bfloat16)
                            out_ap = scr[:]
                        nc.scalar.activation(
                            out_ap, ps[:], mybir.ActivationFunctionType.Relu,
                            bias=tq_t[:, 0:1], scale=1.0,
                            accum_out=hg_t[:, hg_col[u]:hg_col[u] + 1])

            nc.sync.dma_start(rowmax_d[:], rm_t[:])
            nc.sync.dma_start(hinge_d[:], hg_t[:])
    if os.environ.get("MK_DEDUP", "1") == "1":
        _dedup_ldweights(nc, mybir)
    nc.finalize()
    return nc


def _dedup_ldweights(nc, mybir):
    """Remove back-to-back duplicate InstLdweights (same stationary operand).

    The queries are the stationary operand for every matmul; tile_legalize
    emits one InstLdweights per matmul anyway. The PE weight state persists
    across matmuls, so a reload with an identical access pattern is a no-op
    costing ~98ns of PE time each. Only sync-free duplicates are dropped —
    LDWs carrying semaphore waits/updates stay (finalize may hang waits on
    them later, and we must not reorder sync).
    """
    f = nc.m.functions[0]
    for blk in f.blocks:
        keep = []
        last_sig = None
        for inst in blk.instructions:
            if isinstance(inst, mybir.InstLdweights):
                sig = str(inst.ins[0])
                si = inst.sync_info
                has_sync = si is not None and (
                    len(si.on_wait) > 0 or len(si.on_update) > 0)
                if sig == last_sig and not has_sync:
                    continue
                last_sig = sig
            keep.append(inst)
        blk.instructions = keep


def _get_nc():
    key = (SPC, CHUNK, UNIT, N_DVE_OF_32, HINGE_INPLACE, KEYS_MODE)
    if key not in _NC_CACHE:
        _NC_CACHE[key] = _build_nc()
    return _NC_CACHE[key]


# ---------------- host side ----------------

def _host_queries(enc2d, mbp, msp, mep, qw, qb):
    start_enc = enc2d[mbp * T + msp]
    end_enc = enc2d[mbp * T + mep]
    q = np.concatenate([start_enc, end_enc], -1).astype(np.float32) @ qw + qb
    return q.astype(np.float32)


def _estimate_sigma(queries, mem_keys):
    # deterministic spread sample of 256 rows -> per-query score sigma
    samp_rows = np.arange(0, ROWS, ROWS // 256)[:256]
    samp = mem_keys[samp_rows].reshape(-1, KD)          # [16384, KD]
    s = queries @ samp.T.astype(np.float32)
    return s.std(axis=1) + 1e-12


def _selection(queries, mem_keys, t_flag, margin, rowmax_all, act_row_flag):
    """Exact top-32 rows + within-row argmax per query.

    rowmax_all: [NM, ROWS] fp32 approx rowmax in 8*s units (-inf where the
      row was drained via ACT), act_row_flag: [NM, ROWS] bool from hinges.
    """
    flags = act_row_flag.copy()
    flags |= rowmax_all >= (KSCALE * (t_flag - margin))[:, None]

    keys2d = mem_keys.reshape(NSLOTS, KD)
    rows_any = np.nonzero(flags.any(axis=0))[0]
    # exact rescore of the union of flagged rows for ALL queries: one sgemm
    ksub = mem_keys[rows_any].astype(np.float32, copy=False)   # [R, 64, KD]
    R = rows_any.size
    s_sub = queries @ ksub.reshape(R * VPR, KD).T              # [NM, R*64]
    s_sub = s_sub.reshape(NM, R, VPR)
    vals_sub = s_sub.max(-1)                                   # [NM, R]
    wi_sub = s_sub.argmax(-1)                                  # [NM, R]
    fl_sub = flags[:, rows_any]                                # [NM, R]

    top_ids = np.empty((NM, K_TOP), np.int64)
    n_flagged = 0
    n_fallback = 0
    for q in range(NM):
        sel = np.nonzero(fl_sub[q])[0]
        n_flagged += sel.size
        vals = vals_sub[q, sel]
        if sel.size < K_TOP or (vals >= t_flag[q]).sum() < K_TOP:
            # threshold too aggressive for this query -> exact full rescore
            n_fallback += 1
            s = (queries[q] @ keys2d.T).reshape(ROWS, VPR)
            vals = s.max(-1)
            wi = s.argmax(-1)
            order = np.argsort(-vals, kind='stable')[:K_TOP]
            top_ids[q] = order * VPR + wi[order]
        else:
            order = np.argsort(-vals, kind='stable')[:K_TOP]
            rows_q = rows_any[sel[order]]
            top_ids[q] = rows_q * VPR + wi_sub[q, sel[order]]
    stats = dict(flagged_rows_per_q=n_flagged / NM, union_rows=int(R),
                 fallback_queries=n_fallback)
    return top_ids, stats


def _tail(enc2d, mbp, msp, mask, mem_keys, queries, top_ids, uw, ub, g, bb):
    keys2d = mem_keys.reshape(NSLOTS, KD)
    top_keys = keys2d[top_ids]                           # [NM, K, KD]
    s = np.einsum('qd,qkd->qk', queries, top_keys).astype(np.float32)
    s = s - s.max(-1, keepdims=True)
    e = np.exp(s)
    attn = e / e.sum(-1, keepdims=True)
    retrieved = np.einsum('qk,qkd->qd', attn, top_keys).astype(np.float32)
    retrieved *= mask[:, None]
    update = retrieved @ uw + ub
    upd = enc2d.copy()
    np.add.at(upd, mbp * T + msp, update)
    mu = upd.mean(-1, keepdims=True)
    var = ((upd - mu) ** 2).mean(-1, keepdims=True)
    out = (upd - mu) / np.sqrt(var + LN_EPS) * g + bb
    return out.astype(np.float32).reshape(B, T, H)


def _prep_in_maps(mem_keys, queries, tqneg):
    kdt = bf16 if KEYS_MODE == "bf16" else f8e4
    qdt = f8e4 if KEYS_MODE == "f8mm" else bf16
    keys2d_s = (mem_keys.reshape(NSLOTS, KD) * KSCALE).astype(kdt)
    qT = np.ascontiguousarray(queries.T).astype(qdt)
    in_maps = []
    for c in range(N_CORES):
        shard = np.ascontiguousarray(keys2d_s[c * SPC:(c + 1) * SPC].T)
        in_maps.append({"keysT": shard, "qT": qT, "tqneg": tqneg})
    return in_maps


def run_full(inputs, trace=False, trace_cores=None):
    _maybe_patch_ldwopt()
    from concourse.bass_utils import run_bass_kernel_spmd

    enc = np.asarray(inputs['encoded_input'], np.float32)
    mbp = np.asarray(inputs['mention_batch_positions']).astype(np.int64)
    msp = np.asarray(inputs['mention_start_positions']).astype(np.int64)
    mep = np.asarray(inputs['mention_end_positions']).astype(np.int64)
    mask = np.asarray(inputs['mention_mask'], np.float32)
    mem_keys = np.asarray(inputs['memory_keys'], np.float32)
    qw = np.asarray(inputs['query_w'], np.float32)
    qb = np.asarray(inputs['query_b'], np.float32)
    uw = np.asarray(inputs['update_w'], np.float32)
    ub = np.asarray(inputs['update_b'], np.float32)
    g = np.asarray(inputs['ln_gamma'], np.float32)
    bb = np.asarray(inputs['ln_beta'], np.float32)

    enc2d = enc.reshape(B * T, H)
    queries = _host_queries(enc2d, mbp, msp, mep, qw, qb)
    sigma = _estimate_sigma(queries, mem_keys)
    t_flag = (Z_FLAG * sigma).astype(np.float32)
    margin = (M_Z * sigma).astype(np.float32)
    # device hinge fires when 8*s - (8*t_flag - 8*margin) > 0
    tqneg = (-(KSCALE * (t_flag - margin)))[:, None].astype(np.float32)
    in_maps = _prep_in_maps(mem_keys, queries, tqneg)

    nc = _get_nc()
    res = run_bass_kernel_spmd(nc, in_maps, list(range(N_CORES)),
                               trace=trace, trace_cores=trace_cores)

    # unpack per-core packed outputs
    dve_set = _dve_units(NUNITS, N_DVE_OF_32)
    dve_list = sorted(dve_set)
    act_list = [u for u in range(NUNITS) if u not in dve_set]
    n_dve, n_act = len(dve_list), len(act_list)

    rowmax_all = np.full((NM, ROWS), -np.inf, np.float32)
    act_row_flag = np.zeros((NM, ROWS), bool)
    for c in range(N_CORES):
        if n_dve:
            rm = res.results[c]["rowmax"].reshape(NM, n_dve, ROWS_PER_UNIT)
            for j, u in enumerate(dve_list):
                r0 = c * RPC + u * ROWS_PER_UNIT
                rowmax_all[:, r0:r0 + ROWS_PER_UNIT] = rm[:, j]
        if n_act:
            hg = res.results[c]["hinge"]                 # [NM, n_act]
            pos = np.nan_to_num(hg, nan=1.0, posinf=1.0) > 0
            for j, u in enumerate(act_list):
                r0 = c * RPC + u * ROWS_PER_UNIT
                act_row_flag[:, r0:r0 + ROWS_PER_UNIT] |= pos[:, j:j + 1]

    top_ids, stats = _selection(queries, mem_keys, t_flag, margin,
                                rowmax_all, act_row_flag)
    out = _tail(enc2d, mbp, msp, mask, mem_keys, queries, top_ids, uw, ub, g, bb)
    return out, res, stats


def kernel(**inputs) -> np.ndarray:
    out, _, _ = run_full(inputs, trace=False)
    return out


# revision 23
# speedup vs baseline: 1.0223x; 1.0223x over previous
"""Trainium2 kernel for nn_MemoryAttentionLayer (retrieval_knn) — v2.

Strategy (shard memory rows across 8 cores, replicate queries):

Device (8 cores, SPMD — the full 34.4 GFLOP score scan):
  - each core holds a row-shard of memory_keys as keysT [kd=128, slots]
    in FP8 e4m3 (scaled x8) — halves HBM traffic vs bf16 (48us vs 96us
    at 358 GB/s/core); fp8 matmuls stream at the same 1 col/cycle.
  - PE: stationary qT [kd, 128 queries] fp8 (+~4us HAM warmup burst),
    256 matmuls N=512 -> scores (x8 scale) in PSUM fp32, grouped into
    128 units of 1024 slots = 2 PSUM banks. Four units in flight with
    explicit round-robin slot tags (the pool's default MRU slot reuse
    would serialize each engine into consume->refill->consume).
    Redundant per-matmul LDWEIGHTS are deduped post-build.
  - drain is split between the only two engines that can read PSUM
    (GPSIMD/DMA physically cannot; TT with 2 PSUM operands is illegal):
      * DVE units (17/32): reduce_max over v=64 -> rowmax[q, row]
        (row-granular flags), (1024+140)/0.96 ~ 1.21us per unit.
      * ACT units (15/32): activation(Relu, bias=-(8*t_f - margin),
        accum_out) + accumulator readout -> hinge[q, unit] > 0 iff the
        unit has a slot above threshold (16-row flags), ~1.41us per unit.
    ACT output goes to an SBUF scratch tile: pointing it at PSUM slows
    concurrent matmuls ~300x (measured).
Host (tail, exact):
  - query projection, per-query sigma -> flag threshold t_f = Z*sigma,
  - union of flagged rows -> ONE sgemm exact fp32 rescore for all
    queries x union-rows, per-query top-32 with count-check fallback
    (full rescore) so top-k is exact regardless of fp8/threshold error,
  - attention softmax over 32, update matmul, scatter-add, layer norm.
"""

import os
import numpy as np
import ml_dtypes

bf16 = ml_dtypes.bfloat16
f8e4 = ml_dtypes.float8_e4m3

# ---- problem constants (hardcoded per spec) ----
N_CORES = 8
B, T, H = 4, 512, 768
NM = 128                      # n_mentions / queries
ROWS, VPR, KD = 16384, 64, 128
K_TOP = 32
LN_EPS = 1e-12

NSLOTS = ROWS * VPR           # 1048576
SPC = NSLOTS // N_CORES       # 131072 slots per core
RPC = ROWS // N_CORES         # 2048 rows per core
MMN = 512                     # moving cols per matmul (1 fp32 PSUM bank)
UNIT = int(os.environ.get("MK_UNIT", 1024))  # slots per drain unit
NUNITS = SPC // UNIT          # units per core
ROWS_PER_UNIT = UNIT // VPR
KSCALE = 8.0                  # fp8 key scale; PSUM scores are 8*s

# ---- tunables ----
CHUNK = int(os.environ.get("MK_CHUNK", 16384))      # slots per DMA chunk
N_DVE_OF_32 = int(os.environ.get("MK_NDVE", 17))    # DVE units per 32
Z_FLAG = float(os.environ.get("MK_Z", 3.8))         # flag threshold z
M_Z = float(os.environ.get("MK_MZ", 0.30))          # fp8 margin, in sigmas
HINGE_INPLACE = os.environ.get("MK_HIP", "0") == "1"
# key path: f8cast = fp8 HBM + DMA-cast to bf16 SBUF; bf16 = bf16 HBM;
# f8mix = bf16 queries x fp8 keys matmul; f8mm = fp8 x fp8 matmul
KEYS_MODE = os.environ.get("MK_KEYS", "f8mm")
LDW_OPT = os.environ.get("MK_LDWOPT", "0") == "1"


def _maybe_patch_ldwopt():
    """Flip walrus --enable-ldw-opt when MK_LDWOPT=1 (A/B experiment)."""
    if not LDW_OPT:
        return
    import concourse.bass_utils as _bu
    if getattr(_bu, "_mk_ldwopt_patch", False):
        return
    _orig_rc = _bu.run_command

    def _rc(cmd, **kw):
        cmd = [("--enable-ldw-opt=true" if c == "--enable-ldw-opt=false" else c)
               for c in cmd]
        return _orig_rc(cmd, **kw)

    _bu.run_command = _rc
    _bu._mk_ldwopt_patch = True

_NC_CACHE: dict = {}


def _dve_units(nunits, n_dve_of_32):
    """Which unit indices drain via DVE reduce_max (rest via ACT hinge)."""
    return {u for u in range(nunits)
            if (u * n_dve_of_32) % 32 < n_dve_of_32}


def _build_nc(spc=SPC, chunk=CHUNK, n_dve_of_32=None, hinge_inplace=None):
    import concourse.bacc as bacc
    import concourse.mybir as mybir
    from concourse import tile

    if n_dve_of_32 is None:
        n_dve_of_32 = N_DVE_OF_32
    if hinge_inplace is None:
        hinge_inplace = HINGE_INPLACE

    nunits = spc // UNIT
    mm_per_unit = UNIT // MMN                      # 4
    # ramped chunk plan: tiny first chunk so the PE starts ~5us earlier
    # (a full 16K-slot fp8 chunk is ~6us of DMA before the first matmul).
    chunks = []
    pos = 0
    for sz in (2048, 6144, 8192):
        if pos + sz <= spc and sz <= chunk:
            chunks.append((pos, sz))
            pos += sz
    while pos < spc:
        sz = min(chunk, spc - pos)
        chunks.append((pos, sz))
        pos += sz
    dve_set = _dve_units(nunits, n_dve_of_32)
    n_dve = len(dve_set)
    n_act = nunits - n_dve

    mode = KEYS_MODE
    kdt_dram = (mybir.dt.bfloat16 if mode == "bf16" else mybir.dt.float8e4)
    kdt_sbuf = (mybir.dt.float8e4 if mode in ("f8mm", "f8mix")
                else mybir.dt.bfloat16)
    qdt = mybir.dt.float8e4 if mode == "f8mm" else mybir.dt.bfloat16

    nc = bacc.Bacc()
    keysT_d = nc.dram_tensor("keysT", [KD, spc], kdt_dram,
                             kind="ExternalInput")
    qT_d = nc.dram_tensor("qT", [KD, NM], qdt,
                          kind="ExternalInput")
    tqneg_d = nc.dram_tensor("tqneg", [NM, 1], mybir.dt.float32,
                             kind="ExternalInput")
    rowmax_d = nc.dram_tensor("rowmax", [NM, max(n_dve, 1) * ROWS_PER_UNIT],
                              mybir.dt.float32, kind="ExternalOutput")
    hinge_d = nc.dram_tensor("hinge", [NM, max(n_act, 1)],
                             mybir.dt.float32, kind="ExternalOutput")

    # packed output columns per unit
    rm_col = {}
    hg_col = {}
    for u in range(nunits):
        if u in dve_set:
            rm_col[u] = len(rm_col)
        else:
            hg_col[u] = len(hg_col)

    with tile.TileContext(nc) as tc:
        with (
            tc.tile_pool(name="kpool", bufs=3) as kpool,
            tc.tile_pool(name="const", bufs=1) as const_pool,
            tc.tile_pool(name="outs", bufs=1) as out_pool,
            tc.tile_pool(name="scr", bufs=2) as scr_pool,
            tc.tile_pool(name="ps", bufs=1, space="PSUM") as ps_pool,
        ):
            q_t = const_pool.tile([KD, NM], qdt)
            nc.sync.dma_start(q_t[:], qT_d[:])
            tq_t = const_pool.tile([NM, 1], mybir.dt.float32)
            nc.sync.dma_start(tq_t[:], tqneg_d[:])

            rm_t = out_pool.tile([NM, max(n_dve, 1) * ROWS_PER_UNIT],
                                 mybir.dt.float32)
            hg_t = out_pool.tile([NM, max(n_act, 1)], mybir.dt.float32)

            # consume the tq DMA dep up front (keeps scheduler simple)
            tq_warm = const_pool.tile([NM, 1], mybir.dt.float32)
            nc.scalar.copy(tq_warm[:], tq_t[:])

            # No HAM warmup burst: with the ramped first chunks the first
            # real fill is ready at ~8.5us, and a warmup parked on any PSUM
            # slot delays that unit's fill past 12us — measured as a net
            # loss. Cold-clock fills (375ns vs 216ns warm) have slack since
            # the drain engines set the pace.
            if n_act == 0:
                nc.scalar.memzero(hg_t[:])
            if n_dve == 0:
                nc.scalar.memzero(rm_t[:])

            nslots = 8 * MMN // UNIT
            for cstart, csize in chunks:
                k_t = kpool.tile([KD, csize], kdt_sbuf, tag="k_t")
                if mode == "f8cast":
                    nc.gpsimd.dma_start(
                        k_t[:], keysT_d[:, cstart:cstart + csize])
                else:
                    nc.sync.dma_start(
                        k_t[:], keysT_d[:, cstart:cstart + csize])
                for ui in range(csize // UNIT):
                    u = (cstart + ui * UNIT) // UNIT
                    # explicit slot tag: strict round-robin over the PSUM
                    # banks so each engine's next unit is pre-filled while
                    # it processes the current one (the pool's default slot
                    # reuse is most-recently-freed, which serializes
                    # consume -> refill -> consume per engine).
                    ps = ps_pool.tile([NM, UNIT], mybir.dt.float32,
                                      tag=f"ps{u % nslots}")
                    for mi in range(mm_per_unit):
                        off = ui * UNIT + mi * MMN
                        nc.tensor.matmul(ps[:, mi * MMN:(mi + 1) * MMN],
                                         q_t[:],
                                         k_t[:, off:off + MMN],
                                         start=True, stop=True)
                    if u in dve_set:
                        o0 = rm_col[u] * ROWS_PER_UNIT
                        nc.vector.reduce_max(
                            rm_t[:, o0:o0 + ROWS_PER_UNIT],
                            ps[:].rearrange("p (r v) -> p r v", v=VPR),
                            axis=mybir.AxisListType.X)
                    else:
                        if hinge_inplace:
                            out_ap = ps[:]
                        else:
                            scr = scr_pool.tile([NM, UNIT], mybir.dt.bfloat16)
                            out_ap = scr[:]
                        nc.scalar.activation(
                            out_ap, ps[:], mybir.ActivationFunctionType.Relu,
                            bias=tq_t[:, 0:1], scale=1.0,
                            accum_out=hg_t[:, hg_col[u]:hg_col[u] + 1])

            nc.sync.dma_start(rowmax_d[:], rm_t[:])
            nc.sync.dma_start(hinge_d[:], hg_t[:])
    if os.environ.get("MK_DEDUP", "1") == "1":
        _dedup_ldweights(nc, mybir)
    nc.finalize()
    return nc


def _dedup_ldweights(nc, mybir):
    """Remove back-to-back duplicate InstLdweights (same stationary operand).

    The queries are the stationary operand for every matmul; tile_legalize
    emits one InstLdweights per matmul anyway. The PE weight state persists
    across matmuls, so a reload with an identical access pattern is a no-op
    costing ~98ns of PE time each. Only sync-free duplicates are dropped —
    LDWs carrying semaphore waits/updates stay (finalize may hang waits on
    them later, and we must not reorder sync).
    """
    f = nc.m.functions[0]
    for blk in f.blocks:
        keep = []
        last_sig = None
        for inst in blk.instructions:
            if isinstance(inst, mybir.InstLdweights):
                sig = str(inst.ins[0])
                si = inst.sync_info
                has_sync = si is not None and (
                    len(si.on_wait) > 0 or len(si.on_update) > 0)
                if sig == last_sig and not has_sync:
                    continue
                last_sig = sig
            keep.append(inst)
        blk.instructions = keep


def _get_nc():
    key = (SPC, CHUNK, UNIT, N_DVE_OF_32, HINGE_INPLACE, KEYS_MODE)
    if key not in _NC_CACHE:
        _NC_CACHE[key] = _build_nc()
    return _NC_CACHE[key]


# ---------------- host side ----------------

def _host_queries(enc2d, mbp, msp, mep, qw, qb):
    start_enc = enc2d[mbp * T + msp]
    end_enc = enc2d[mbp * T + mep]
    q = np.concatenate([start_enc, end_enc], -1).astype(np.float32) @ qw + qb
    return q.astype(np.float32)


def _estimate_sigma(queries, mem_keys):
    # deterministic spread sample of 256 rows -> per-query score sigma
    samp_rows = np.arange(0, ROWS, ROWS // 256)[:256]
    samp = mem_keys[samp_rows].reshape(-1, KD)          # [16384, KD]
    s = queries @ samp.T.astype(np.float32)
    return s.std(axis=1) + 1e-12


def _selection(queries, mem_keys, t_flag, margin, rowmax_all, act_row_flag):
    """Exact top-32 rows + within-row argmax per query.

    rowmax_all: [NM, ROWS] fp32 approx rowmax in 8*s units (-inf where the
      row was drained via ACT), act_row_flag: [NM, ROWS] bool from hinges.
    """
    flags = act_row_flag.copy()
    flags |= rowmax_all >= (KSCALE * (t_flag - margin))[:, None]

    keys2d = mem_keys.reshape(NSLOTS, KD)
    rows_any = np.nonzero(flags.any(axis=0))[0]
    # exact rescore of the union of flagged rows for ALL queries: one sgemm
    ksub = mem_keys[rows_any].astype(np.float32, copy=False)   # [R, 64, KD]
    R = rows_any.size
    s_sub = queries @ ksub.reshape(R * VPR, KD).T              # [NM, R*64]
    s_sub = s_sub.reshape(NM, R, VPR)
    vals_sub = s_sub.max(-1)                                   # [NM, R]
    wi_sub = s_sub.argmax(-1)                                  # [NM, R]
    fl_sub = flags[:, rows_any]                                # [NM, R]

    top_ids = np.empty((NM, K_TOP), np.int64)
    n_flagged = 0
    n_fallback = 0
    for q in range(NM):
        sel = np.nonzero(fl_sub[q])[0]
        n_flagged += sel.size
        vals = vals_sub[q, sel]
        if sel.size < K_TOP or (vals >= t_flag[q]).sum() < K_TOP:
            # threshold too aggressive for this query -> exact full rescore
            n_fallback += 1
            s = (queries[q] @ keys2d.T).reshape(ROWS, VPR)
            vals = s.max(-1)
            wi = s.argmax(-1)
            order = np.argsort(-vals, kind='stable')[:K_TOP]
            top_ids[q] = order * VPR + wi[order]
        else:
            order = np.argsort(-vals, kind='stable')[:K_TOP]
            rows_q = rows_any[sel[order]]
            top_ids[q] = rows_q * VPR + wi_sub[q, sel[order]]
    stats = dict(flagged_rows_per_q=n_flagged / NM, union_rows=int(R),
                 fallback_queries=n_fallback)
    return top_ids, stats


def _tail(enc2d, mbp, msp, mask, mem_keys, queries, top_ids, uw, ub, g, bb):
    keys2d = mem_keys.reshape(NSLOTS, KD)
    top_keys = keys2d[top_ids]                           # [NM, K, KD]
    s = np.einsum('qd,qkd->qk', queries, top_keys).astype(np.float32)
    s = s - s.max(-1, keepdims=True)
    e = np.exp(s)
    attn = e / e.sum(-1, keepdims=True)
    retrieved = np.einsum('qk,qkd->qd', attn, top_keys).astype(np.float32)
    retrieved *= mask[:, None]
    update = retrieved @ uw + ub
    upd = enc2d.copy()
    np.add.at(upd, mbp * T + msp, update)
    mu = upd.mean(-1, keepdims=True)
    var = ((upd - mu) ** 2).mean(-1, keepdims=True)
    out = (upd - mu) / np.sqrt(var + LN_EPS) * g + bb
    return out.astype(np.float32).reshape(B, T, H)


def _prep_in_maps(mem_keys, queries, tqneg):
    kdt = bf16 if KEYS_MODE == "bf16" else f8e4
    qdt = f8e4 if KEYS_MODE == "f8mm" else bf16
    keys2d_s = (mem_keys.reshape(NSLOTS, KD) * KSCALE).astype(kdt)
    qT = np.ascontiguousarray(queries.T).astype(qdt)
    in_maps = []
    for c in range(N_CORES):
        shard = np.ascontiguousarray(keys2d_s[c * SPC:(c + 1) * SPC].T)
        in_maps.append({"keysT": shard, "qT": qT, "tqneg": tqneg})
    return in_maps


def run_full(inputs, trace=False, trace_cores=None):
    _maybe_patch_ldwopt()
    from concourse.bass_utils import run_bass_kernel_spmd

    enc = np.asarray(inputs['encoded_input'], np.float32)
    mbp = np.asarray(inputs['mention_batch_positions']).astype(np.int64)
    msp = np.asarray(inputs['mention_start_positions']).astype(np.int64)
    mep = np.asarray(inputs['mention_end_positions']).astype(np.int64)
    mask = np.asarray(inputs['mention_mask'], np.float32)
    mem_keys = np.asarray(inputs['memory_keys'], np.float32)
    qw = np.asarray(inputs['query_w'], np.float32)
    qb = np.asarray(inputs['query_b'], np.float32)
    uw = np.asarray(inputs['update_w'], np.float32)
    ub = np.asarray(inputs['update_b'], np.float32)
    g = np.asarray(inputs['ln_gamma'], np.float32)
    bb = np.asarray(inputs['ln_beta'], np.float32)

    enc2d = enc.reshape(B * T, H)
    queries = _host_queries(enc2d, mbp, msp, mep, qw, qb)
    sigma = _estimate_sigma(queries, mem_keys)
    t_flag = (Z_FLAG * sigma).astype(np.float32)
    margin = (M_Z * sigma).astype(np.float32)
    # device hinge fires when 8*s - (8*t_flag - 8*margin) > 0
    tqneg = (-(KSCALE * (t_flag - margin)))[:, None].astype(np.float32)
    in_maps = _prep_in_maps(mem_keys, queries, tqneg)

    nc = _get_nc()
    res = run_bass_kernel_spmd(nc, in_maps, list(range(N_CORES)),
                               trace=trace, trace_cores=trace_cores)

    # unpack per-core packed outputs
    dve_set = _dve_units(NUNITS, N_DVE_OF_32)
    dve_list = sorted(dve_set)
    act_list = [u for u in range(NUNITS) if u not in dve_set]
    n_dve, n_act = len(dve_list), len(act_list)

    rowmax_all = np.full((NM, ROWS), -np.inf, np.float32)
    act_row_flag = np.zeros((NM, ROWS), bool)
    for c in range(N_CORES):
        if n_dve:
            rm = res.results[c]["rowmax"].reshape(NM, n_dve, ROWS_PER_UNIT)
            for j, u in enumerate(dve_list):
                r0 = c * RPC + u * ROWS_PER_UNIT
                rowmax_all[:, r0:r0 + ROWS_PER_UNIT] = rm[:, j]
        if n_act:
            hg = res.results[c]["hinge"]                 # [NM, n_act]
            pos = np.nan_to_num(hg, nan=1.0, posinf=1.0) > 0
            for j, u in enumerate(act_list):
                r0 = c * RPC + u * ROWS_PER_UNIT
                act_row_flag[:, r0:r0 + ROWS_PER_UNIT] |= pos[:, j:j + 1]

    top_ids, stats = _selection(queries, mem_keys, t_flag, margin,
                                rowmax_all, act_row_flag)
    out = _tail(enc2d, mbp, msp, mask, mem_keys, queries, top_ids, uw, ub, g, bb)
    return out, res, stats


def kernel(**inputs) -> np.ndarray:
    out, _, _ = run_full(inputs, trace=False)
    return out
